# revision 24
# baseline (speedup 1.0000x reference)
"""Causal self-attention (B=4, T=2048, C=1024, 16 heads x d=64) on 8 trn2 NeuronCores.

Strategy: tensor-parallel over heads — core i owns heads (2i, 2i+1).
Everything on-device runs in feature-major ("transposed") layout:
  xT [C, B*T] (host pre-transposes once) ->
  qT/kT [128=2*64 feats, T] per batch, vT -> V via PE transpose,
  S^T = K Q^T blocks [128 k, 512 q] (row-packed: both heads concurrently),
  P^T = exp(S^T/8) with causal zeroing via affine_select,
  y^T [65, 512] = [V | ones]^T P^T  (ones column makes row 64 the softmax
  denominator, accumulated over k-blocks in PSUM),
  normalize with an outer-product broadcast of 1/denominator,
  out^T partial [C, B*T] = W_proj_rows^T y^T, DMA'd straight from PSUM.
Host sums the 8 partial projections and adds b_proj.
"""

import sys

if "/opt/trn_rl_repo" not in sys.path:
    sys.path.insert(0, "/opt/trn_rl_repo")

import contextlib
import ctypes
import types

import numpy as np

import concourse.bass as bass
import concourse.mybir as mybir
import concourse.tile as tile
from concourse.bass_utils import run_bass_kernel_spmd

B, T, C = 4, 2048, 1024
N_HEAD, D = 16, 64
NCORES = 8
F32 = mybir.dt.float32

# matmul operand dtype: "fp32" (bit-exact, 4 cyc/row) or "fp32r" (1 cyc/row at
# free-dim >= 256, reduced-precision PE read)
DT_MM = "fp32r"
TRACE = False  # test.py flips this for profiled runs

_SO_PATH = "/opt/axon/libaxon_pjrt.so"


# ---------------------------------------------------------------------------
# Environment shims: (1) register the NTFF profile hook trn_boot could not
# install (image's antenv lacks axon_hooks); (2) this walrus build caps sem
# waits per instruction, but Tile's tail drain carries one wait per active
# proc — spread them over single-wait SP NOPs instead.
# ---------------------------------------------------------------------------
def _install_ntff_hook():
    if "antenv.axon_hooks" in sys.modules:
        return
    state = {"hook": None}

    def set_hook(h):
        state["hook"] = h

    def get_hook():
        return state["hook"]

    mod = types.ModuleType("antenv.axon_hooks")
    mod.set_axon_ntff_profile_hook = set_hook
    mod.get_axon_ntff_profile_hook = get_hook
    sys.modules["antenv.axon_hooks"] = mod
    import antenv

    antenv.axon_hooks = mod

    try:
        lib = ctypes.CDLL(_SO_PATH)
    except OSError:
        return
    if not hasattr(lib, "axon_start_nrt_profile"):
        return
    lib.axon_start_nrt_profile.argtypes = [
        ctypes.POINTER(ctypes.c_int64),
        ctypes.c_size_t,
    ]
    lib.axon_start_nrt_profile.restype = ctypes.c_int64
    lib.axon_stop_nrt_profile.argtypes = [ctypes.c_char_p]
    lib.axon_stop_nrt_profile.restype = ctypes.c_int64

    @contextlib.contextmanager
    def _hook_cm(output_dir, device_ids):
        import jax

        jax.devices()
        if device_ids:
            ids = (ctypes.c_int64 * len(device_ids))(*device_ids)
            rc = lib.axon_start_nrt_profile(ids, len(device_ids))
        else:
            rc = lib.axon_start_nrt_profile(None, 0)
        if rc != 0:
            raise RuntimeError(f"axon_start_nrt_profile rc={rc}")
        try:
            yield
        finally:
            n = lib.axon_stop_nrt_profile(str(output_dir).encode())
            if n < 0:
                raise RuntimeError(f"axon_stop_nrt_profile rc={n}")
            print(f"profile: {n} file(s) written to {output_dir}", file=sys.stderr)

    set_hook(_hook_cm)


def _patch_tile_tail_drain():
    from concourse.vector_clock import ScopedClock, VectorClock

    if getattr(tile.TileContext, "_drain_patch", False):
        return

    def patched(self, tick_clock, wait_clock):
        vc = tick_clock.global_clock
        n = len(vc)
        for proc in range(n):
            t = vc[proc]
            if t <= 0:
                continue
            sub = VectorClock([t if i == proc else 0 for i in range(n)])
            nop = self.nc.sync.nop(nofuse=True)
            wait_clock.add_sem_waits(nop.ins, ScopedClock({None: sub}))
        # Same tail as the original _drain_and_barrier, minus the multi-wait
        # drain — the NOP chain above already waited on every proc.
        self.nc.sync.drain()
        self.nc.all_engine_barrier()
        assert self.sems is not None
        popped = self.nc._tile_sem_poison_stack.pop()
        assert popped is self._sem_poison
        self.nc.clear_and_free_semaphores(list(self.sems.allocated().values()))
        self.nc.all_engine_barrier()

    tile.TileContext._drain_and_barrier = patched
    tile.TileContext._drain_patch = True


_install_ntff_hook()
_patch_tile_tail_drain()


def _split_waits(nc, limit=1):
    """This walrus build rejects instructions carrying more than ~2 sem waits.
    Spill excess waits onto preceding same-engine NOPs (program order on the
    issuing engine preserves the blocking semantics exactly)."""
    k = 0
    for fn in nc.m.functions:
        for bb in fn.blocks:
            new = []
            for ins in bb.instructions:
                si = ins.sync_info
                waits = list(si.on_wait) if si and si.on_wait else []
                if len(waits) > limit:
                    for w in waits[:-limit]:
                        nop = mybir.InstNoOp(name=f"I-wsplit-{k}")
                        k += 1
                        nop.engine = ins.engine
                        nop.sync_info = mybir.SyncInfo(on_wait=[w], on_update=[])
                        new.append(nop)
                    ins.sync_info = mybir.SyncInfo(
                        on_wait=waits[-limit:],
                        on_update=list(si.on_update) if si.on_update else [],
                    )
                new.append(ins)
            bb.instructions = new


def _op_dtype():
    return {
        "fp32": mybir.dt.float32,
        "fp32r": mybir.dt.float32r,
        "bf16": mybir.dt.bfloat16,
    }[DT_MM]


def _op_npdtype():
    return mybir.dt.np(_op_dtype())


def build_nc():
    DT = _op_dtype()
    nc = bass.Bass()
    xT = nc.declare_dram_parameter("xT", [C, B * T], DT, isOutput=False)
    wqkv = nc.declare_dram_parameter("wqkv", [C, 384], DT, isOutput=False)
    bqkv = nc.declare_dram_parameter("bqkv", [128, 3], F32, isOutput=False)
    wproj = nc.declare_dram_parameter("wproj", [128, C], DT, isOutput=False)
    ident = nc.declare_dram_parameter("ident", [128, 128], DT, isOutput=False)
    outT = nc.declare_dram_parameter("outT", [C, B * T], F32, isOutput=True)

    EXP = mybir.ActivationFunctionType.Exp

    wide = mybir.dt.size(DT) > 2  # debug dtypes need smaller pools to fit SBUF
    with tile.TileContext(nc) as tc:
        with contextlib.ExitStack() as ctx:
            singles = ctx.enter_context(tc.tile_pool(name="singles", bufs=1))
            xpool = ctx.enter_context(tc.tile_pool(name="xpool", bufs=10 if wide else 16))
            qkv_sb = ctx.enter_context(tc.tile_pool(name="qkv_sb", bufs=2))
            vt_pool = ctx.enter_context(tc.tile_pool(name="vtp", bufs=1))
            vaug_p = ctx.enter_context(tc.tile_pool(name="vaug", bufs=2))
            pt_pool = ctx.enter_context(tc.tile_pool(name="ptp", bufs=2 if wide else 3))
            yt_pool = ctx.enter_context(tc.tile_pool(name="ytp", bufs=1 if wide else 2))
            yub_p = ctx.enter_context(tc.tile_pool(name="yub", bufs=1 if wide else 2))
            sm_pool = ctx.enter_context(tc.tile_pool(name="smp", bufs=2))
            rb_pool = ctx.enter_context(tc.tile_pool(name="rbp", bufs=1 if wide else 2))
            ost_pool = ctx.enter_context(tc.tile_pool(name="ost", bufs=2 if wide else 3))
            dscr = ctx.enter_context(tc.tile_pool(name="dscr", bufs=2, space="DRAM"))
            # PSUM (8 banks): s [128,2,512] x2 = 4, y0+y1 [65,512] x1 = 2,
            # ps1 [128,512] x2 = 2 (qkv accum / transposes / proj out)
            ps1 = ctx.enter_context(tc.tile_pool(name="ps1", bufs=2, space="PSUM"))
            ps_s = ctx.enter_context(tc.tile_pool(name="ps_s", bufs=2, space="PSUM"))
            ps_y = ctx.enter_context(tc.tile_pool(name="ps_y", bufs=1, space="PSUM"))

            wq_sb = singles.tile([128, 8, 384], DT)
            nc.sync.dma_start(out=wq_sb, in_=wqkv.rearrange("(a p) f -> p a f", p=128))
            wp_sb = singles.tile([128, C], DT)
            nc.sync.dma_start(out=wp_sb, in_=wproj[:, :])
            bq_sb = singles.tile([128, 3], F32)
            nc.sync.dma_start(out=bq_sb, in_=bqkv[:, :])
            id_sb = singles.tile([128, 128], DT)
            nc.sync.dma_start(out=id_sb, in_=ident[:, :])

            state = {}

            def emit_qkv(b):
                qT = qkv_sb.tile([128, T], DT, tag="qT")
                kT = qkv_sb.tile([128, T], DT, tag="kT")
                vT = vt_pool.tile([128, T], DT, tag="vT")
                for tch in range(4):
                    slabs = []
                    for c in range(8):
                        sl = xpool.tile([128, 512], DT, tag="xslab")
                        nc.sync.dma_start(
                            out=sl,
                            in_=xT[
                                c * 128 : (c + 1) * 128,
                                b * T + tch * 512 : b * T + (tch + 1) * 512,
                            ],
                        )
                        slabs.append(sl)
                    for m, dst in enumerate((qT, kT, vT)):
                        ps = ps1.tile([128, 512], F32, tag="ps1")
                        for c in range(8):
                            nc.tensor.matmul(
                                ps,
                                lhsT=wq_sb[:, c, m * 128 : (m + 1) * 128],
                                rhs=slabs[c],
                                start=(c == 0),
                                stop=(c == 7),
                            )
                        nc.vector.tensor_scalar_add(
                            dst[:, tch * 512 : (tch + 1) * 512], ps, bq_sb[:, m : m + 1]
                        )
                # vT -> V (token-major) + ones column
                va0 = vaug_p.tile([128, 16, 65], DT, tag="va0")
                va1 = vaug_p.tile([128, 16, 65], DT, tag="va1")
                nc.vector.tensor_copy(va0[:, :, 64:65], ones_col)
                nc.vector.tensor_copy(va1[:, :, 64:65], ones_col)
                for tt in range(16):
                    tp = ps1.tile([128, 128], DT, tag="ps1")
                    nc.tensor.transpose(tp, vT[:, tt * 128 : (tt + 1) * 128], id_sb)
                    nc.vector.tensor_copy(va0[:, tt, 0:64], tp[:, 0:64])
                    nc.vector.tensor_copy(va1[:, tt, 0:64], tp[:, 64:128])
                state[b] = {"qT": qT, "kT": kT, "va0": va0, "va1": va1}

            def emit_attention(b):
                st = state[b]
                qT, kT, va0, va1 = st["qT"], st["kT"], st["va0"], st["va1"]
                yub = yub_p.tile([65, 8, 512], F32, tag="yub")
                tmp2s = []
                for qc in range(4):
                    y0 = ps_y.tile([65, 512], F32, tag="y0")
                    y1 = ps_y.tile([65, 512], F32, tag="y1")
                    nkb = 4 * qc + 4
                    for kb in range(nkb):
                        s = ps_s.tile([128, 2, 512], F32, tag="s")
                        nc.tensor.matmul(
                            s[:, 0, :],
                            lhsT=kT[0:64, kb * 128 : (kb + 1) * 128],
                            rhs=qT[0:64, qc * 512 : (qc + 1) * 512],
                            start=True,
                            stop=True,
                            tile_position=(0, 0),
                        )
                        nc.tensor.matmul(
                            s[:, 1, :],
                            lhsT=kT[64:128, kb * 128 : (kb + 1) * 128],
                            rhs=qT[64:128, qc * 512 : (qc + 1) * 512],
                            start=True,
                            stop=True,
                            tile_position=(64, 0),
                        )
                        pt = pt_pool.tile([128, 2, 512], DT, tag="pt")
                        # for diagonal blocks only columns q >= (kb-4qc)*128
                        # are causally reachable; skip the rest entirely
                        j = max(kb - 4 * qc, 0) if kb >= 4 * qc else 0
                        lo = j * 128
                        nc.scalar.activation(pt[:, :, lo:512], s[:, :, lo:512], EXP, scale=0.125)
                        if kb >= 4 * qc:
                            nc.gpsimd.affine_select(
                                out=pt[:, :, lo : lo + 128],
                                in_=pt[:, :, lo : lo + 128],
                                pattern=[[0, 2], [1, 128]],
                                base=0,
                                channel_multiplier=-1,
                                compare_op=mybir.AluOpType.is_ge,
                                fill=0.0,
                            )
                        nc.tensor.matmul(
                            y0[:, lo:512],
                            lhsT=va0[:, kb, :],
                            rhs=pt[:, 0, lo:512],
                            start=(kb == 0),
                            stop=(kb == nkb - 1),
                        )
                        nc.tensor.matmul(
                            y1[:, lo:512],
                            lhsT=va1[:, kb, :],
                            rhs=pt[:, 1, lo:512],
                            start=(kb == 0),
                            stop=(kb == nkb - 1),
                        )
                    # release y psum quickly; stash denominators on partition 0
                    nc.vector.tensor_copy(yub[:, 2 * qc, :], y0[:, :])
                    nc.vector.tensor_copy(yub[:, 2 * qc + 1, :], y1[:, :])
                # batch-level: one DMA re-partition bounce + one 8-lane recip
                tmp8 = sm_pool.tile([1, 8, 512], F32, tag="tmp8")
                for r in range(8):
                    nc.scalar.copy(tmp8[:, r, :], yub[64:65, r, :])
                dsc = dscr.tile([1, 8, 512], F32, tag="dsc")
                nc.sync.dma_start(out=dsc, in_=tmp8)
                sums8 = sm_pool.tile([8, 512], F32, tag="sums8")
                nc.sync.dma_start(out=sums8, in_=dsc.rearrange("o h q -> (o h) q"))
                r8 = sm_pool.tile([8, 512], F32, tag="r8")
                nc.vector.reciprocal(r8, sums8)
                dsc2 = dscr.tile([8, 512], F32, tag="dsc2")
                nc.sync.dma_start(out=dsc2, in_=r8)
                st["yub"] = yub
                st["dsc2"] = dsc2

            def emit_finish(b):
                st = state.pop(b)
                yub, dsc2 = st["yub"], st["dsc2"]
                # broadcast 1/denominator rows across 64 partitions via DMA
                rbs = rb_pool.tile([64, 8, 512], F32, tag="rbs")
                for r in range(8):
                    row = dsc2[r : r + 1, :]
                    bcast = bass.AP(
                        tensor=row.tensor,
                        offset=row.offset,
                        ap=[[0, 64]] + [p for p in row.ap if p[1] != 1],
                    )
                    nc.sync.dma_start(out=rbs[:, r, :], in_=bcast)
                yT = yt_pool.tile([128, T], DT, tag="yT")
                for qc in range(4):
                    nc.vector.tensor_mul(
                        yT[0:64, qc * 512 : (qc + 1) * 512],
                        yub[0:64, 2 * qc, :],
                        rbs[:, 2 * qc, :],
                    )
                    nc.vector.tensor_mul(
                        yT[64:128, qc * 512 : (qc + 1) * 512],
                        yub[0:64, 2 * qc + 1, :],
                        rbs[:, 2 * qc + 1, :],
                    )
                for mt in range(8):
                    for tch in range(4):
                        o = ps1.tile([128, 512], F32, tag="ps1")
                        nc.tensor.matmul(
                            o,
                            lhsT=wp_sb[:, mt * 128 : (mt + 1) * 128],
                            rhs=yT[:, tch * 512 : (tch + 1) * 512],
                            start=True,
                            stop=True,
                        )
                        osb = ost_pool.tile([128, 512], F32, tag="osb")
                        nc.vector.tensor_copy(osb, o)
                        nc.sync.dma_start(
                            out=outT[
                                mt * 128 : (mt + 1) * 128,
                                b * T + tch * 512 : b * T + (tch + 1) * 512,
                            ],
                            in_=osb,
                        )

            ones_col = singles.tile([128, 16, 1], F32)
            nc.vector.memset(ones_col, 1.0)

            for b in range(B):
                emit_qkv(b)
                emit_attention(b)
                if b > 0:
                    emit_finish(b - 1)
            emit_finish(B - 1)

    _split_waits(nc)
    return nc


_nc_cache = None


def kernel(x, W_qkv, b_qkv, W_proj, b_proj):
    global _nc_cache
    x = np.ascontiguousarray(np.asarray(x, dtype=np.float32))
    W_qkv = np.asarray(W_qkv, dtype=np.float32)
    b_qkv = np.asarray(b_qkv, dtype=np.float32)
    W_proj = np.asarray(W_proj, dtype=np.float32)
    b_proj = np.asarray(b_proj, dtype=np.float32)

    npdt = _op_npdtype()
    xT = np.ascontiguousarray(x.reshape(B * T, C).T).astype(npdt)
    ident = np.eye(128, dtype=np.float32).astype(npdt)

    in_maps = []
    for i in range(NCORES):
        s = slice(128 * i, 128 * (i + 1))
        wq = np.ascontiguousarray(
            np.concatenate(
                [W_qkv[:, s], W_qkv[:, 1024:2048][:, s], W_qkv[:, 2048:3072][:, s]],
                axis=1,
            )
        ).astype(npdt)
        bq = np.ascontiguousarray(
            np.stack([b_qkv[0:1024][s], b_qkv[1024:2048][s], b_qkv[2048:3072][s]], axis=1)
        )
        wp = np.ascontiguousarray(W_proj[s, :]).astype(npdt)
        in_maps.append(
            {"xT": xT, "wqkv": wq, "bqkv": bq, "wproj": wp, "ident": ident}
        )

    if _nc_cache is None:
        _nc_cache = build_nc()
    res = run_bass_kernel_spmd(_nc_cache, in_maps, list(range(NCORES)), trace=TRACE)
    kernel.last_result = res

    acc = np.zeros((C, B * T), dtype=np.float32)
    for r in res.results:
        acc += r["outT"]
    out = acc.T.reshape(B, T, C) + b_proj
    return out.astype(np.float32)


# revision 26
# speedup vs baseline: 1.0125x; 1.0125x over previous
"""Causal self-attention (B=4, T=2048, C=1024, 16 heads x d=64) on 8 trn2 NeuronCores.

Strategy: tensor-parallel over heads — core i owns heads (2i, 2i+1).
Everything on-device runs in feature-major ("transposed") layout:
  xT [C, B*T] (host pre-transposes once) ->
  qT/kT [128=2*64 feats, T] per batch, vT -> V via PE transpose,
  S^T = K Q^T blocks [128 k, 512 q] (row-packed: both heads concurrently),
  P^T = exp(S^T/8) with causal zeroing via affine_select,
  y^T [65, 512] = [V | ones]^T P^T  (ones column makes row 64 the softmax
  denominator, accumulated over k-blocks in PSUM),
  normalize with an outer-product broadcast of 1/denominator,
  out^T partial [C, B*T] = W_proj_rows^T y^T, DMA'd straight from PSUM.
Host sums the 8 partial projections and adds b_proj.
"""

import sys

if "/opt/trn_rl_repo" not in sys.path:
    sys.path.insert(0, "/opt/trn_rl_repo")

import contextlib
import ctypes
import types

import numpy as np

import concourse.bass as bass
import concourse.mybir as mybir
import concourse.tile as tile
from concourse.bass_utils import run_bass_kernel_spmd

B, T, C = 4, 2048, 1024
N_HEAD, D = 16, 64
NCORES = 8
F32 = mybir.dt.float32

# matmul operand dtype: "fp32" (bit-exact, 4 cyc/row) or "fp32r" (1 cyc/row at
# free-dim >= 256, reduced-precision PE read)
DT_MM = "fp32r"
TRACE = False  # test.py flips this for profiled runs

_SO_PATH = "/opt/axon/libaxon_pjrt.so"


# ---------------------------------------------------------------------------
# Environment shims: (1) register the NTFF profile hook trn_boot could not
# install (image's antenv lacks axon_hooks); (2) this walrus build caps sem
# waits per instruction, but Tile's tail drain carries one wait per active
# proc — spread them over single-wait SP NOPs instead.
# ---------------------------------------------------------------------------
def _install_ntff_hook():
    if "antenv.axon_hooks" in sys.modules:
        return
    state = {"hook": None}

    def set_hook(h):
        state["hook"] = h

    def get_hook():
        return state["hook"]

    mod = types.ModuleType("antenv.axon_hooks")
    mod.set_axon_ntff_profile_hook = set_hook
    mod.get_axon_ntff_profile_hook = get_hook
    sys.modules["antenv.axon_hooks"] = mod
    import antenv

    antenv.axon_hooks = mod

    try:
        lib = ctypes.CDLL(_SO_PATH)
    except OSError:
        return
    if not hasattr(lib, "axon_start_nrt_profile"):
        return
    lib.axon_start_nrt_profile.argtypes = [
        ctypes.POINTER(ctypes.c_int64),
        ctypes.c_size_t,
    ]
    lib.axon_start_nrt_profile.restype = ctypes.c_int64
    lib.axon_stop_nrt_profile.argtypes = [ctypes.c_char_p]
    lib.axon_stop_nrt_profile.restype = ctypes.c_int64

    @contextlib.contextmanager
    def _hook_cm(output_dir, device_ids):
        import jax

        jax.devices()
        if device_ids:
            ids = (ctypes.c_int64 * len(device_ids))(*device_ids)
            rc = lib.axon_start_nrt_profile(ids, len(device_ids))
        else:
            rc = lib.axon_start_nrt_profile(None, 0)
        if rc != 0:
            raise RuntimeError(f"axon_start_nrt_profile rc={rc}")
        try:
            yield
        finally:
            n = lib.axon_stop_nrt_profile(str(output_dir).encode())
            if n < 0:
                raise RuntimeError(f"axon_stop_nrt_profile rc={n}")
            print(f"profile: {n} file(s) written to {output_dir}", file=sys.stderr)

    set_hook(_hook_cm)


def _patch_tile_tail_drain():
    from concourse.vector_clock import ScopedClock, VectorClock

    if getattr(tile.TileContext, "_drain_patch", False):
        return

    def patched(self, tick_clock, wait_clock):
        vc = tick_clock.global_clock
        n = len(vc)
        for proc in range(n):
            t = vc[proc]
            if t <= 0:
                continue
            sub = VectorClock([t if i == proc else 0 for i in range(n)])
            nop = self.nc.sync.nop(nofuse=True)
            wait_clock.add_sem_waits(nop.ins, ScopedClock({None: sub}))
        # Same tail as the original _drain_and_barrier, minus the multi-wait
        # drain — the NOP chain above already waited on every proc.
        self.nc.sync.drain()
        self.nc.all_engine_barrier()
        assert self.sems is not None
        popped = self.nc._tile_sem_poison_stack.pop()
        assert popped is self._sem_poison
        self.nc.clear_and_free_semaphores(list(self.sems.allocated().values()))
        self.nc.all_engine_barrier()

    tile.TileContext._drain_and_barrier = patched
    tile.TileContext._drain_patch = True


_install_ntff_hook()
_patch_tile_tail_drain()


def _split_waits(nc, limit=1):
    """This walrus build rejects instructions carrying more than ~2 sem waits.
    Spill excess waits onto preceding same-engine NOPs (program order on the
    issuing engine preserves the blocking semantics exactly)."""
    k = 0
    for fn in nc.m.functions:
        for bb in fn.blocks:
            new = []
            for ins in bb.instructions:
                si = ins.sync_info
                waits = list(si.on_wait) if si and si.on_wait else []
                if len(waits) > limit:
                    for w in waits[:-limit]:
                        nop = mybir.InstNoOp(name=f"I-wsplit-{k}")
                        k += 1
                        nop.engine = ins.engine
                        nop.sync_info = mybir.SyncInfo(on_wait=[w], on_update=[])
                        new.append(nop)
                    ins.sync_info = mybir.SyncInfo(
                        on_wait=waits[-limit:],
                        on_update=list(si.on_update) if si.on_update else [],
                    )
                new.append(ins)
            bb.instructions = new


def _op_dtype():
    return {
        "fp32": mybir.dt.float32,
        "fp32r": mybir.dt.float32r,
        "bf16": mybir.dt.bfloat16,
    }[DT_MM]


def _op_npdtype():
    return mybir.dt.np(_op_dtype())


def build_nc():
    DT = _op_dtype()
    nc = bass.Bass()
    xT = nc.declare_dram_parameter("xT", [C, B * T], DT, isOutput=False)
    wqkv = nc.declare_dram_parameter("wqkv", [C, 384], DT, isOutput=False)
    bqkv = nc.declare_dram_parameter("bqkv", [128, 3], F32, isOutput=False)
    wproj = nc.declare_dram_parameter("wproj", [128, C], DT, isOutput=False)
    ident = nc.declare_dram_parameter("ident", [128, 128], DT, isOutput=False)
    outT = nc.declare_dram_parameter("outT", [C, B * T], F32, isOutput=True)

    EXP = mybir.ActivationFunctionType.Exp

    wide = mybir.dt.size(DT) > 2  # debug dtypes need smaller pools to fit SBUF
    with tile.TileContext(nc) as tc:
        with contextlib.ExitStack() as ctx:
            singles = ctx.enter_context(tc.tile_pool(name="singles", bufs=1))
            xpool = ctx.enter_context(tc.tile_pool(name="xpool", bufs=10 if wide else 16))
            qkv_sb = ctx.enter_context(tc.tile_pool(name="qkv_sb", bufs=2))
            vt_pool = ctx.enter_context(tc.tile_pool(name="vtp", bufs=1))
            vaug_p = ctx.enter_context(tc.tile_pool(name="vaug", bufs=2))
            pt_pool = ctx.enter_context(tc.tile_pool(name="ptp", bufs=2 if wide else 3))
            yt_pool = ctx.enter_context(tc.tile_pool(name="ytp", bufs=1 if wide else 2))
            yub_p = ctx.enter_context(tc.tile_pool(name="yub", bufs=1 if wide else 2))
            sm_pool = ctx.enter_context(tc.tile_pool(name="smp", bufs=2))
            rb_pool = ctx.enter_context(tc.tile_pool(name="rbp", bufs=1 if wide else 2))
            ost_pool = ctx.enter_context(tc.tile_pool(name="ost", bufs=2 if wide else 3))
            dscr = ctx.enter_context(tc.tile_pool(name="dscr", bufs=2, space="DRAM"))
            # PSUM (8 banks): s [128,2,512] x2 = 4, y0+y1 [65,512] x1 = 2,
            # ps1 [128,512] x2 = 2 (qkv accum / transposes / proj out)
            ps1 = ctx.enter_context(tc.tile_pool(name="ps1", bufs=2, space="PSUM"))
            ps_s = ctx.enter_context(tc.tile_pool(name="ps_s", bufs=2, space="PSUM"))
            ps_y = ctx.enter_context(tc.tile_pool(name="ps_y", bufs=1, space="PSUM"))

            wq_sb = singles.tile([128, 8, 384], DT)
            nc.sync.dma_start(out=wq_sb, in_=wqkv.rearrange("(a p) f -> p a f", p=128))
            wp_sb = singles.tile([128, C], DT)
            nc.sync.dma_start(out=wp_sb, in_=wproj[:, :])
            bq_sb = singles.tile([128, 3], F32)
            nc.sync.dma_start(out=bq_sb, in_=bqkv[:, :])
            id_sb = singles.tile([128, 128], DT)
            nc.sync.dma_start(out=id_sb, in_=ident[:, :])

            state = {}

            def emit_qkv(b):
                qT = qkv_sb.tile([128, T], DT, tag="qT")
                kT = qkv_sb.tile([128, T], DT, tag="kT")
                vT = vt_pool.tile([128, T], DT, tag="vT")
                for tch in range(4):
                    slabs = []
                    for c in range(8):
                        sl = xpool.tile([128, 512], DT, tag="xslab")
                        nc.sync.dma_start(
                            out=sl,
                            in_=xT[
                                c * 128 : (c + 1) * 128,
                                b * T + tch * 512 : b * T + (tch + 1) * 512,
                            ],
                        )
                        slabs.append(sl)
                    for m, dst in enumerate((qT, kT, vT)):
                        ps = ps1.tile([128, 512], F32, tag="ps1")
                        for c in range(8):
                            nc.tensor.matmul(
                                ps,
                                lhsT=wq_sb[:, c, m * 128 : (m + 1) * 128],
                                rhs=slabs[c],
                                start=(c == 0),
                                stop=(c == 7),
                            )
                        nc.vector.tensor_scalar_add(
                            dst[:, tch * 512 : (tch + 1) * 512], ps, bq_sb[:, m : m + 1]
                        )
                # vT -> V (token-major) + ones column
                va0 = vaug_p.tile([128, 16, 65], DT, tag="va0")
                va1 = vaug_p.tile([128, 16, 65], DT, tag="va1")
                nc.vector.tensor_copy(va0[:, :, 64:65], ones_col)
                nc.vector.tensor_copy(va1[:, :, 64:65], ones_col)
                for tt in range(16):
                    tp = ps1.tile([128, 128], DT, tag="ps1")
                    nc.tensor.transpose(tp, vT[:, tt * 128 : (tt + 1) * 128], id_sb)
                    nc.vector.tensor_copy(va0[:, tt, 0:64], tp[:, 0:64])
                    nc.vector.tensor_copy(va1[:, tt, 0:64], tp[:, 64:128])
                state[b] = {"qT": qT, "kT": kT, "va0": va0, "va1": va1}

            def emit_attention(b, per_qc_finish=False):
                st = state[b]
                qT, kT, va0, va1 = st["qT"], st["kT"], st["va0"], st["va1"]
                yub = yub_p.tile([65, 8, 512], F32, tag="yub")
                if per_qc_finish:
                    yTq_t = yt_pool.tile([128, T], DT, tag="yT")
                    st["yTq"] = yTq_t
                for qc in range(4):
                    y0 = ps_y.tile([65, 512], F32, tag="y0")
                    y1 = ps_y.tile([65, 512], F32, tag="y1")
                    nkb = 4 * qc + 4
                    for kb in range(nkb):
                        s = ps_s.tile([128, 2, 512], F32, tag="s")
                        nc.tensor.matmul(
                            s[:, 0, :],
                            lhsT=kT[0:64, kb * 128 : (kb + 1) * 128],
                            rhs=qT[0:64, qc * 512 : (qc + 1) * 512],
                            start=True,
                            stop=True,
                        )
                        nc.tensor.matmul(
                            s[:, 1, :],
                            lhsT=kT[64:128, kb * 128 : (kb + 1) * 128],
                            rhs=qT[64:128, qc * 512 : (qc + 1) * 512],
                            start=True,
                            stop=True,
                        )
                        pt = pt_pool.tile([128, 2, 512], DT, tag="pt")
                        # for diagonal blocks only columns q >= (kb-4qc)*128
                        # are causally reachable; skip the rest entirely
                        j = max(kb - 4 * qc, 0) if kb >= 4 * qc else 0
                        lo = j * 128
                        nc.scalar.activation(pt[:, :, lo:512], s[:, :, lo:512], EXP, scale=0.125)
                        if kb >= 4 * qc:
                            nc.gpsimd.affine_select(
                                out=pt[:, :, lo : lo + 128],
                                in_=pt[:, :, lo : lo + 128],
                                pattern=[[0, 2], [1, 128]],
                                base=0,
                                channel_multiplier=-1,
                                compare_op=mybir.AluOpType.is_ge,
                                fill=0.0,
                            )
                        nc.tensor.matmul(
                            y0[:, lo:512],
                            lhsT=va0[:, kb, :],
                            rhs=pt[:, 0, lo:512],
                            start=(kb == 0),
                            stop=(kb == nkb - 1),
                        )
                        nc.tensor.matmul(
                            y1[:, lo:512],
                            lhsT=va1[:, kb, :],
                            rhs=pt[:, 1, lo:512],
                            start=(kb == 0),
                            stop=(kb == nkb - 1),
                        )
                    # release y psum quickly; stash denominators on partition 0
                    nc.vector.tensor_copy(yub[:, 2 * qc, :], y0[:, :])
                    nc.vector.tensor_copy(yub[:, 2 * qc + 1, :], y1[:, :])
                    if per_qc_finish:
                        tmp2 = sm_pool.tile([1, 2, 512], F32, tag="tmp2")
                        nc.scalar.copy(tmp2[:, 0, :], yub[64:65, 2 * qc, :])
                        nc.scalar.copy(tmp2[:, 1, :], yub[64:65, 2 * qc + 1, :])
                        dscq = dscr.tile([1, 2, 512], F32, tag="dscq")
                        nc.sync.dma_start(out=dscq, in_=tmp2)
                        s2q = sm_pool.tile([2, 512], F32, tag="s2q")
                        nc.sync.dma_start(out=s2q, in_=dscq.rearrange("o h q -> (o h) q"))
                        r2q = sm_pool.tile([2, 512], F32, tag="r2q")
                        nc.vector.reciprocal(r2q, s2q)
                        d2q = dscr.tile([2, 512], F32, tag="d2q")
                        nc.sync.dma_start(out=d2q, in_=r2q)
                        rbq = rb_pool.tile([64, 2, 512], F32, tag="rbq")
                        for h in range(2):
                            row = d2q[h : h + 1, :]
                            bcast = bass.AP(
                                tensor=row.tensor,
                                offset=row.offset,
                                ap=[[0, 64]] + [p for p in row.ap if p[1] != 1],
                            )
                            nc.sync.dma_start(out=rbq[:, h, :], in_=bcast)
                        yTq = state[b]["yTq"]
                        nc.vector.tensor_mul(
                            yTq[0:64, qc * 512 : (qc + 1) * 512],
                            yub[0:64, 2 * qc, :],
                            rbq[:, 0, :],
                        )
                        nc.vector.tensor_mul(
                            yTq[64:128, qc * 512 : (qc + 1) * 512],
                            yub[0:64, 2 * qc + 1, :],
                            rbq[:, 1, :],
                        )
                        emit_proj_chunk(b, yTq, qc)
                if per_qc_finish:
                    state.pop(b)
                    return
                # batch-level: one DMA re-partition bounce + one 8-lane recip
                tmp8 = sm_pool.tile([1, 8, 512], F32, tag="tmp8")
                for r in range(8):
                    nc.scalar.copy(tmp8[:, r, :], yub[64:65, r, :])
                dsc = dscr.tile([1, 8, 512], F32, tag="dsc")
                nc.sync.dma_start(out=dsc, in_=tmp8)
                sums8 = sm_pool.tile([8, 512], F32, tag="sums8")
                nc.sync.dma_start(out=sums8, in_=dsc.rearrange("o h q -> (o h) q"))
                r8 = sm_pool.tile([8, 512], F32, tag="r8")
                nc.vector.reciprocal(r8, sums8)
                dsc2 = dscr.tile([8, 512], F32, tag="dsc2")
                nc.sync.dma_start(out=dsc2, in_=r8)
                st["yub"] = yub
                st["dsc2"] = dsc2

            def emit_proj_chunk(b, yT, tch):
                for mt in range(8):
                    o = ps1.tile([128, 512], F32, tag="ps1")
                    nc.tensor.matmul(
                        o,
                        lhsT=wp_sb[:, mt * 128 : (mt + 1) * 128],
                        rhs=yT[:, tch * 512 : (tch + 1) * 512],
                        start=True,
                        stop=True,
                    )
                    osb = ost_pool.tile([128, 512], F32, tag="osb")
                    nc.vector.tensor_copy(osb, o)
                    nc.sync.dma_start(
                        out=outT[
                            mt * 128 : (mt + 1) * 128,
                            b * T + tch * 512 : b * T + (tch + 1) * 512,
                        ],
                        in_=osb,
                    )

            def emit_finish(b):
                st = state.pop(b)
                yub, dsc2 = st["yub"], st["dsc2"]
                # broadcast 1/denominator rows across 64 partitions via DMA
                rbs = rb_pool.tile([64, 8, 512], F32, tag="rbs")
                for r in range(8):
                    row = dsc2[r : r + 1, :]
                    bcast = bass.AP(
                        tensor=row.tensor,
                        offset=row.offset,
                        ap=[[0, 64]] + [p for p in row.ap if p[1] != 1],
                    )
                    nc.sync.dma_start(out=rbs[:, r, :], in_=bcast)
                yT = yt_pool.tile([128, T], DT, tag="yT")
                for qc in range(4):
                    nc.vector.tensor_mul(
                        yT[0:64, qc * 512 : (qc + 1) * 512],
                        yub[0:64, 2 * qc, :],
                        rbs[:, 2 * qc, :],
                    )
                    nc.vector.tensor_mul(
                        yT[64:128, qc * 512 : (qc + 1) * 512],
                        yub[0:64, 2 * qc + 1, :],
                        rbs[:, 2 * qc + 1, :],
                    )
                for tch in range(4):
                    emit_proj_chunk(b, yT, tch)

            ones_col = singles.tile([128, 16, 1], F32)
            nc.vector.memset(ones_col, 1.0)

            for b in range(B):
                emit_qkv(b)
                emit_attention(b, per_qc_finish=(b == B - 1))
                if b > 0 and b - 1 < B - 1:
                    emit_finish(b - 1)

    _split_waits(nc)
    return nc


_nc_cache = None


def kernel(x, W_qkv, b_qkv, W_proj, b_proj):
    global _nc_cache
    x = np.ascontiguousarray(np.asarray(x, dtype=np.float32))
    W_qkv = np.asarray(W_qkv, dtype=np.float32)
    b_qkv = np.asarray(b_qkv, dtype=np.float32)
    W_proj = np.asarray(W_proj, dtype=np.float32)
    b_proj = np.asarray(b_proj, dtype=np.float32)

    npdt = _op_npdtype()
    xT = np.ascontiguousarray(x.reshape(B * T, C).T).astype(npdt)
    ident = np.eye(128, dtype=np.float32).astype(npdt)

    in_maps = []
    for i in range(NCORES):
        s = slice(128 * i, 128 * (i + 1))
        wq = np.ascontiguousarray(
            np.concatenate(
                [W_qkv[:, s], W_qkv[:, 1024:2048][:, s], W_qkv[:, 2048:3072][:, s]],
                axis=1,
            )
        ).astype(npdt)
        bq = np.ascontiguousarray(
            np.stack([b_qkv[0:1024][s], b_qkv[1024:2048][s], b_qkv[2048:3072][s]], axis=1)
        )
        wp = np.ascontiguousarray(W_proj[s, :]).astype(npdt)
        in_maps.append(
            {"xT": xT, "wqkv": wq, "bqkv": bq, "wproj": wp, "ident": ident}
        )

    if _nc_cache is None:
        _nc_cache = build_nc()
    res = run_bass_kernel_spmd(_nc_cache, in_maps, list(range(NCORES)), trace=TRACE)
    kernel.last_result = res

    acc = np.zeros((C, B * T), dtype=np.float32)
    for r in res.results:
        acc += r["outT"]
    out = acc.T.reshape(B, T, C) + b_proj
    return out.astype(np.float32)


# revision 27
# speedup vs baseline: 1.0379x; 1.0251x over previous
"""Causal self-attention (B=4, T=2048, C=1024, 16 heads x d=64) on 8 trn2 NeuronCores.

Strategy: tensor-parallel over heads — core i owns heads (2i, 2i+1).
Everything on-device runs in feature-major ("transposed") layout:
  xT [C, B*T] (host pre-transposes once) ->
  qT/kT [128=2*64 feats, T] per batch, vT -> V via PE transpose,
  S^T = K Q^T blocks [128 k, 512 q] (row-packed: both heads concurrently),
  P^T = exp(S^T/8) with causal zeroing via affine_select,
  y^T [65, 512] = [V | ones]^T P^T  (ones column makes row 64 the softmax
  denominator, accumulated over k-blocks in PSUM),
  normalize with an outer-product broadcast of 1/denominator,
  out^T partial [C, B*T] = W_proj_rows^T y^T, DMA'd straight from PSUM.
Host sums the 8 partial projections and adds b_proj.
"""

import sys

if "/opt/trn_rl_repo" not in sys.path:
    sys.path.insert(0, "/opt/trn_rl_repo")

import contextlib
import ctypes
import types

import numpy as np

import concourse.bass as bass
import concourse.mybir as mybir
import concourse.tile as tile
from concourse.bass_utils import run_bass_kernel_spmd

B, T, C = 4, 2048, 1024
N_HEAD, D = 16, 64
NCORES = 8
F32 = mybir.dt.float32

# matmul operand dtype: "fp32" (bit-exact, 4 cyc/row) or "fp32r" (1 cyc/row at
# free-dim >= 256, reduced-precision PE read)
DT_MM = "fp32r"
TRACE = False  # test.py flips this for profiled runs

_SO_PATH = "/opt/axon/libaxon_pjrt.so"


# ---------------------------------------------------------------------------
# Environment shims: (1) register the NTFF profile hook trn_boot could not
# install (image's antenv lacks axon_hooks); (2) this walrus build caps sem
# waits per instruction, but Tile's tail drain carries one wait per active
# proc — spread them over single-wait SP NOPs instead.
# ---------------------------------------------------------------------------
def _install_ntff_hook():
    if "antenv.axon_hooks" in sys.modules:
        return
    state = {"hook": None}

    def set_hook(h):
        state["hook"] = h

    def get_hook():
        return state["hook"]

    mod = types.ModuleType("antenv.axon_hooks")
    mod.set_axon_ntff_profile_hook = set_hook
    mod.get_axon_ntff_profile_hook = get_hook
    sys.modules["antenv.axon_hooks"] = mod
    import antenv

    antenv.axon_hooks = mod

    try:
        lib = ctypes.CDLL(_SO_PATH)
    except OSError:
        return
    if not hasattr(lib, "axon_start_nrt_profile"):
        return
    lib.axon_start_nrt_profile.argtypes = [
        ctypes.POINTER(ctypes.c_int64),
        ctypes.c_size_t,
    ]
    lib.axon_start_nrt_profile.restype = ctypes.c_int64
    lib.axon_stop_nrt_profile.argtypes = [ctypes.c_char_p]
    lib.axon_stop_nrt_profile.restype = ctypes.c_int64

    @contextlib.contextmanager
    def _hook_cm(output_dir, device_ids):
        import jax

        jax.devices()
        if device_ids:
            ids = (ctypes.c_int64 * len(device_ids))(*device_ids)
            rc = lib.axon_start_nrt_profile(ids, len(device_ids))
        else:
            rc = lib.axon_start_nrt_profile(None, 0)
        if rc != 0:
            raise RuntimeError(f"axon_start_nrt_profile rc={rc}")
        try:
            yield
        finally:
            n = lib.axon_stop_nrt_profile(str(output_dir).encode())
            if n < 0:
                raise RuntimeError(f"axon_stop_nrt_profile rc={n}")
            print(f"profile: {n} file(s) written to {output_dir}", file=sys.stderr)

    set_hook(_hook_cm)


def _patch_tile_tail_drain():
    from concourse.vector_clock import ScopedClock, VectorClock

    if getattr(tile.TileContext, "_drain_patch", False):
        return

    def patched(self, tick_clock, wait_clock):
        vc = tick_clock.global_clock
        n = len(vc)
        for proc in range(n):
            t = vc[proc]
            if t <= 0:
                continue
            sub = VectorClock([t if i == proc else 0 for i in range(n)])
            nop = self.nc.sync.nop(nofuse=True)
            wait_clock.add_sem_waits(nop.ins, ScopedClock({None: sub}))
        # Same tail as the original _drain_and_barrier, minus the multi-wait
        # drain — the NOP chain above already waited on every proc.
        self.nc.sync.drain()
        self.nc.all_engine_barrier()
        assert self.sems is not None
        popped = self.nc._tile_sem_poison_stack.pop()
        assert popped is self._sem_poison
        self.nc.clear_and_free_semaphores(list(self.sems.allocated().values()))
        self.nc.all_engine_barrier()

    tile.TileContext._drain_and_barrier = patched
    tile.TileContext._drain_patch = True


_install_ntff_hook()
_patch_tile_tail_drain()


def _split_waits(nc, limit=1):
    """This walrus build rejects instructions carrying more than ~2 sem waits.
    Spill excess waits onto preceding same-engine NOPs (program order on the
    issuing engine preserves the blocking semantics exactly)."""
    k = 0
    for fn in nc.m.functions:
        for bb in fn.blocks:
            new = []
            for ins in bb.instructions:
                si = ins.sync_info
                waits = list(si.on_wait) if si and si.on_wait else []
                if len(waits) > limit:
                    for w in waits[:-limit]:
                        nop = mybir.InstNoOp(name=f"I-wsplit-{k}")
                        k += 1
                        nop.engine = ins.engine
                        nop.sync_info = mybir.SyncInfo(on_wait=[w], on_update=[])
                        new.append(nop)
                    ins.sync_info = mybir.SyncInfo(
                        on_wait=waits[-limit:],
                        on_update=list(si.on_update) if si.on_update else [],
                    )
                new.append(ins)
            bb.instructions = new


def _op_dtype():
    return {
        "fp32": mybir.dt.float32,
        "fp32r": mybir.dt.float32r,
        "bf16": mybir.dt.bfloat16,
    }[DT_MM]


def _op_npdtype():
    return mybir.dt.np(_op_dtype())


def build_nc():
    DT = _op_dtype()
    nc = bass.Bass()
    xT = nc.declare_dram_parameter("xT", [C, B * T], DT, isOutput=False)
    wqkv = nc.declare_dram_parameter("wqkv", [C, 384], DT, isOutput=False)
    bqkv = nc.declare_dram_parameter("bqkv", [128, 3], F32, isOutput=False)
    wproj = nc.declare_dram_parameter("wproj", [128, C], DT, isOutput=False)
    ident = nc.declare_dram_parameter("ident", [128, 128], DT, isOutput=False)
    outT = nc.declare_dram_parameter("outT", [C, B * T], F32, isOutput=True)

    EXP = mybir.ActivationFunctionType.Exp

    wide = mybir.dt.size(DT) > 2  # debug dtypes need smaller pools to fit SBUF
    with tile.TileContext(nc) as tc:
        with contextlib.ExitStack() as ctx:
            singles = ctx.enter_context(tc.tile_pool(name="singles", bufs=1))
            xpool = ctx.enter_context(tc.tile_pool(name="xpool", bufs=10 if wide else 16))
            qkv_sb = ctx.enter_context(tc.tile_pool(name="qkv_sb", bufs=2))
            vt_pool = ctx.enter_context(tc.tile_pool(name="vtp", bufs=1))
            vaug_p = ctx.enter_context(tc.tile_pool(name="vaug", bufs=2))
            pt_pool = ctx.enter_context(tc.tile_pool(name="ptp", bufs=2 if wide else 3))
            yt_pool = ctx.enter_context(tc.tile_pool(name="ytp", bufs=1 if wide else 2))
            yub_p = ctx.enter_context(tc.tile_pool(name="yub", bufs=1 if wide else 2))
            sm_pool = ctx.enter_context(tc.tile_pool(name="smp", bufs=2))
            rb_pool = ctx.enter_context(tc.tile_pool(name="rbp", bufs=1 if wide else 2))
            ost_pool = ctx.enter_context(tc.tile_pool(name="ost", bufs=2 if wide else 3))
            dscr = ctx.enter_context(tc.tile_pool(name="dscr", bufs=2, space="DRAM"))
            # PSUM (8 banks): s [128,2,512] x2 = 4, y0+y1 [65,512] x1 = 2,
            # ps1 [128,512] x2 = 2 (qkv accum / transposes / proj out)
            ps1 = ctx.enter_context(tc.tile_pool(name="ps1", bufs=2, space="PSUM"))
            ps_s = ctx.enter_context(tc.tile_pool(name="ps_s", bufs=2, space="PSUM"))
            ps_y = ctx.enter_context(tc.tile_pool(name="ps_y", bufs=1, space="PSUM"))

            wq_sb = singles.tile([128, 8, 384], DT)
            nc.sync.dma_start(out=wq_sb, in_=wqkv.rearrange("(a p) f -> p a f", p=128))
            wp_sb = singles.tile([128, C], DT)
            nc.sync.dma_start(out=wp_sb, in_=wproj[:, :])
            bq_sb = singles.tile([128, 3], F32)
            nc.sync.dma_start(out=bq_sb, in_=bqkv[:, :])
            id_sb = singles.tile([128, 128], DT)
            nc.sync.dma_start(out=id_sb, in_=ident[:, :])

            state = {}

            def emit_qkv(b):
                qT = qkv_sb.tile([128, T], DT, tag="qT")
                kT = qkv_sb.tile([128, T], DT, tag="kT")
                vT = vt_pool.tile([128, T], DT, tag="vT")
                for tch in range(4):
                    slabs = []
                    for c in range(8):
                        sl = xpool.tile([128, 512], DT, tag="xslab")
                        nc.sync.dma_start(
                            out=sl,
                            in_=xT[
                                c * 128 : (c + 1) * 128,
                                b * T + tch * 512 : b * T + (tch + 1) * 512,
                            ],
                        )
                        slabs.append(sl)
                    for m, dst in enumerate((qT, kT, vT)):
                        ps = ps1.tile([128, 512], F32, tag="ps1")
                        for c in range(8):
                            nc.tensor.matmul(
                                ps,
                                lhsT=wq_sb[:, c, m * 128 : (m + 1) * 128],
                                rhs=slabs[c],
                                start=(c == 0),
                                stop=(c == 7),
                            )
                        nc.vector.tensor_scalar_add(
                            dst[:, tch * 512 : (tch + 1) * 512], ps, bq_sb[:, m : m + 1]
                        )
                # vT -> V (token-major) + ones column
                va0 = vaug_p.tile([128, 16, 65], DT, tag="va0")
                va1 = vaug_p.tile([128, 16, 65], DT, tag="va1")
                nc.vector.tensor_copy(va0[:, :, 64:65], ones_col)
                nc.vector.tensor_copy(va1[:, :, 64:65], ones_col)
                for tt in range(16):
                    tp = ps1.tile([128, 128], DT, tag="ps1")
                    nc.tensor.transpose(tp, vT[:, tt * 128 : (tt + 1) * 128], id_sb)
                    nc.vector.tensor_copy(va0[:, tt, 0:64], tp[:, 0:64])
                    nc.vector.tensor_copy(va1[:, tt, 0:64], tp[:, 64:128])
                state[b] = {"qT": qT, "kT": kT, "va0": va0, "va1": va1}

            def emit_attention(b, per_qc_finish=False):
                st = state[b]
                qT, kT, va0, va1 = st["qT"], st["kT"], st["va0"], st["va1"]
                yub = yub_p.tile([65, 8, 512], F32, tag="yub")
                if per_qc_finish:
                    yTq_t = yt_pool.tile([128, T], DT, tag="yT")
                    st["yTq"] = yTq_t
                for qc in range(4):
                    y0 = ps_y.tile([65, 512], F32, tag="y0")
                    y1 = ps_y.tile([65, 512], F32, tag="y1")
                    nkb = 4 * qc + 4
                    for kb in range(nkb):
                        s = ps_s.tile([128, 2, 512], F32, tag="s")
                        nc.tensor.matmul(
                            s[:, 0, :],
                            lhsT=kT[0:64, kb * 128 : (kb + 1) * 128],
                            rhs=qT[0:64, qc * 512 : (qc + 1) * 512],
                            start=True,
                            stop=True,
                        )
                        nc.tensor.matmul(
                            s[:, 1, :],
                            lhsT=kT[64:128, kb * 128 : (kb + 1) * 128],
                            rhs=qT[64:128, qc * 512 : (qc + 1) * 512],
                            start=True,
                            stop=True,
                        )
                        pt = pt_pool.tile([128, 2, 512], DT, tag="pt")
                        # for diagonal blocks only columns q >= (kb-4qc)*128
                        # are causally reachable; skip the rest entirely
                        j = max(kb - 4 * qc, 0) if kb >= 4 * qc else 0
                        lo = j * 128
                        nc.scalar.activation(pt[:, :, lo:512], s[:, :, lo:512], EXP, scale=0.125)
                        if kb >= 4 * qc:
                            nc.gpsimd.affine_select(
                                out=pt[:, :, lo : lo + 128],
                                in_=pt[:, :, lo : lo + 128],
                                pattern=[[0, 2], [1, 128]],
                                base=0,
                                channel_multiplier=-1,
                                compare_op=mybir.AluOpType.is_ge,
                                fill=0.0,
                            )
                        nc.tensor.matmul(
                            y0[:, lo:512],
                            lhsT=va0[:, kb, :],
                            rhs=pt[:, 0, lo:512],
                            start=(kb == 0),
                            stop=(kb == nkb - 1),
                        )
                        nc.tensor.matmul(
                            y1[:, lo:512],
                            lhsT=va1[:, kb, :],
                            rhs=pt[:, 1, lo:512],
                            start=(kb == 0),
                            stop=(kb == nkb - 1),
                        )
                    # release y psum quickly; stash denominators on partition 0
                    nc.vector.tensor_copy(yub[:, 2 * qc, :], y0[:, :])
                    nc.vector.tensor_copy(yub[:, 2 * qc + 1, :], y1[:, :])
                    if per_qc_finish:
                        tmp2 = sm_pool.tile([1, 2, 512], F32, tag="tmp2")
                        nc.scalar.copy(tmp2[:, 0, :], yub[64:65, 2 * qc, :])
                        nc.scalar.copy(tmp2[:, 1, :], yub[64:65, 2 * qc + 1, :])
                        dscq = dscr.tile([1, 2, 512], F32, tag="dscq")
                        nc.sync.dma_start(out=dscq, in_=tmp2)
                        s2q = sm_pool.tile([2, 512], F32, tag="s2q")
                        nc.sync.dma_start(out=s2q, in_=dscq.rearrange("o h q -> (o h) q"))
                        r2q = sm_pool.tile([2, 512], F32, tag="r2q")
                        nc.vector.reciprocal(r2q, s2q)
                        d2q = dscr.tile([2, 512], F32, tag="d2q")
                        nc.sync.dma_start(out=d2q, in_=r2q)
                        rbq = rb_pool.tile([64, 2, 512], F32, tag="rbq")
                        for h in range(2):
                            row = d2q[h : h + 1, :]
                            bcast = bass.AP(
                                tensor=row.tensor,
                                offset=row.offset,
                                ap=[[0, 64]] + [p for p in row.ap if p[1] != 1],
                            )
                            nc.sync.dma_start(out=rbq[:, h, :], in_=bcast)
                        yTq = state[b]["yTq"]
                        nc.vector.tensor_mul(
                            yTq[0:64, qc * 512 : (qc + 1) * 512],
                            yub[0:64, 2 * qc, :],
                            rbq[:, 0, :],
                        )
                        nc.vector.tensor_mul(
                            yTq[64:128, qc * 512 : (qc + 1) * 512],
                            yub[0:64, 2 * qc + 1, :],
                            rbq[:, 1, :],
                        )
                        emit_proj_chunk(b, yTq, qc)
                if per_qc_finish:
                    state.pop(b)
                    return
                # batch-level: one DMA re-partition bounce + one 8-lane recip
                tmp8 = sm_pool.tile([1, 8, 512], F32, tag="tmp8")
                for r in range(8):
                    nc.scalar.copy(tmp8[:, r, :], yub[64:65, r, :])
                dsc = dscr.tile([1, 8, 512], F32, tag="dsc")
                nc.sync.dma_start(out=dsc, in_=tmp8)
                sums8 = sm_pool.tile([8, 512], F32, tag="sums8")
                nc.sync.dma_start(out=sums8, in_=dsc.rearrange("o h q -> (o h) q"))
                r8 = sm_pool.tile([8, 512], F32, tag="r8")
                nc.vector.reciprocal(r8, sums8)
                dsc2 = dscr.tile([8, 512], F32, tag="dsc2")
                nc.sync.dma_start(out=dsc2, in_=r8)
                st["yub"] = yub
                st["dsc2"] = dsc2

            def emit_proj_chunk(b, yT, tch):
                for mt in range(8):
                    o = ps1.tile([128, 512], F32, tag="ps1")
                    nc.tensor.matmul(
                        o,
                        lhsT=wp_sb[:, mt * 128 : (mt + 1) * 128],
                        rhs=yT[:, tch * 512 : (tch + 1) * 512],
                        start=True,
                        stop=True,
                    )
                    osb = ost_pool.tile([128, 512], F32, tag="osb")
                    nc.vector.tensor_copy(osb, o)
                    nc.sync.dma_start(
                        out=outT[
                            mt * 128 : (mt + 1) * 128,
                            b * T + tch * 512 : b * T + (tch + 1) * 512,
                        ],
                        in_=osb,
                    )

            def emit_finish(b):
                st = state.pop(b)
                yub, dsc2 = st["yub"], st["dsc2"]
                # broadcast 1/denominator rows across 64 partitions via DMA
                rbs = rb_pool.tile([64, 8, 512], F32, tag="rbs")
                for r in range(8):
                    row = dsc2[r : r + 1, :]
                    bcast = bass.AP(
                        tensor=row.tensor,
                        offset=row.offset,
                        ap=[[0, 64]] + [p for p in row.ap if p[1] != 1],
                    )
                    nc.sync.dma_start(out=rbs[:, r, :], in_=bcast)
                yT = yt_pool.tile([128, T], DT, tag="yT")
                for qc in range(4):
                    nc.vector.tensor_mul(
                        yT[0:64, qc * 512 : (qc + 1) * 512],
                        yub[0:64, 2 * qc, :],
                        rbs[:, 2 * qc, :],
                    )
                    nc.vector.tensor_mul(
                        yT[64:128, qc * 512 : (qc + 1) * 512],
                        yub[0:64, 2 * qc + 1, :],
                        rbs[:, 2 * qc + 1, :],
                    )
                for tch in range(4):
                    emit_proj_chunk(b, yT, tch)

            ones_col = singles.tile([128, 16, 1], F32)
            nc.vector.memset(ones_col, 1.0)

            for b in range(B):
                emit_qkv(b)
                emit_attention(b)
                if b > 0:
                    emit_finish(b - 1)
            emit_finish(B - 1)

    _split_waits(nc)
    return nc


_nc_cache = None


def kernel(x, W_qkv, b_qkv, W_proj, b_proj):
    global _nc_cache
    x = np.ascontiguousarray(np.asarray(x, dtype=np.float32))
    W_qkv = np.asarray(W_qkv, dtype=np.float32)
    b_qkv = np.asarray(b_qkv, dtype=np.float32)
    W_proj = np.asarray(W_proj, dtype=np.float32)
    b_proj = np.asarray(b_proj, dtype=np.float32)

    npdt = _op_npdtype()
    xT = np.ascontiguousarray(x.reshape(B * T, C).T).astype(npdt)
    ident = np.eye(128, dtype=np.float32).astype(npdt)

    in_maps = []
    for i in range(NCORES):
        s = slice(128 * i, 128 * (i + 1))
        wq = np.ascontiguousarray(
            np.concatenate(
                [W_qkv[:, s], W_qkv[:, 1024:2048][:, s], W_qkv[:, 2048:3072][:, s]],
                axis=1,
            )
        ).astype(npdt)
        bq = np.ascontiguousarray(
            np.stack([b_qkv[0:1024][s], b_qkv[1024:2048][s], b_qkv[2048:3072][s]], axis=1)
        )
        wp = np.ascontiguousarray(W_proj[s, :]).astype(npdt)
        in_maps.append(
            {"xT": xT, "wqkv": wq, "bqkv": bq, "wproj": wp, "ident": ident}
        )

    if _nc_cache is None:
        _nc_cache = build_nc()
    res = run_bass_kernel_spmd(_nc_cache, in_maps, list(range(NCORES)), trace=TRACE)
    kernel.last_result = res

    acc = np.zeros((C, B * T), dtype=np.float32)
    for r in res.results:
        acc += r["outT"]
    out = acc.T.reshape(B, T, C) + b_proj
    return out.astype(np.float32)
